# revision 57
# baseline (speedup 1.0000x reference)
"""Trainium2 Bass kernel for an FFM (field-aware factorization machine) layer.

Reference computation (B=16384, P=512, F=16, K=8):
    A[i,j,:] = v[i, f2f[j], :]
    S[i,j]   = sum_k A[i,j,k] * A[j,i,k]          (symmetric)
    rp[b]    = sum_{i<j} x[b,i] * S[i,j] * x[b,j]
    out      = x @ w + rp[:,None] + b

Because S is symmetric, the strictly-upper-triangular quadratic form reduces to
    rp[b] = x[b] @ M @ x[b]^T,   M = 0.5 * (S - diag(S))
so with y' = x @ M + 1*w^T (a plain [512,512] matmul):
    out[b] = sum_j x[b,j] * (y'[b,j]) + bias

Host side folds (v, f2f, w) -> M (a tiny 512x512x8 einsum, ~0.4% of the FLOPs);
the device does the dominant work: the 16384x512x512 matmul, the elementwise
multiply and both reductions, data-parallel over batch across 8 NeuronCores.

Device kernel (per core, batch shard of 2048 rows), transposed orientation:
    x^T tiles produced on-chip: HBM fp32 load -> DVE cast fp16 -> XBAR DMA
    transpose.  y'^T[j,b] accumulated in PSUM from 4 K=128 fp16 matmuls with
    M-chunks stationary; a fused DVE scalar_tensor_tensor computes
    z = (y'^T + w) * x^T; a ones-vector matmul reduces z over partitions into
    rp^T; ACT adds the scalar bias; result DMAs out.
"""

import time
from contextlib import ExitStack

import numpy as np

import concourse.bass as bass
import concourse.mybir as mybir
import concourse.tile as tile
from concourse import bacc
from concourse.bass import ds, ts
from concourse.bass_utils import run_bass_kernel_spmd
from concourse.tile_rust import add_dep_helper


def _raw_inst(bass_inst):
    return getattr(bass_inst, "ins", bass_inst)

B, P, F, K = 16384, 512, 16, 8
N_CORES = 8
B_SH = B // N_CORES          # 2048 batch rows per core
BT = 512                     # batch tile (free dim of transposed tiles)
NBT = B_SH // BT             # 4 batch tiles per core
NC128 = P // 128             # 4 chunks of 128 along the feature dim

FP32 = mybir.dt.float32
FP16 = mybir.dt.float16

# test.py can read this after calling kernel() (exec_time_ns etc.)
LAST_RESULT = None


def _build_nc(bias: float) -> bass.Bass:
    nc = bacc.Bacc("TRN2", target_bir_lowering=False, debug=False,
                   num_devices=N_CORES)

    x_d = nc.dram_tensor("x", [B_SH, P], FP32, kind="ExternalInput")
    # m_d[jc, p, ic, q] = M[ic*128 + p, jc*128 + q]  (fp16, host-prepared)
    m_d = nc.dram_tensor("m", [NC128, 128, NC128, 128], FP16,
                         kind="ExternalInput")
    # w_d[p, c] = w[c*128 + p]
    w_d = nc.dram_tensor("w", [128, NC128], FP32, kind="ExternalInput")
    id_d = nc.dram_tensor("ident", [128, 128], FP16, kind="ExternalInput")
    out_d = nc.dram_tensor("out", [B_SH, 1], FP32, kind="ExternalOutput")

    with tile.TileContext(nc) as tc, ExitStack() as ctx:
        const = ctx.enter_context(tc.tile_pool(name="const", bufs=1))
        xn16p = ctx.enter_context(tc.tile_pool(name="xn16", bufs=3))
        xtp = ctx.enter_context(tc.tile_pool(name="xt", bufs=2))
        zp = ctx.enter_context(tc.tile_pool(name="z", bufs=3))
        orp = ctx.enter_context(tc.tile_pool(name="orow", bufs=2))
        pyp = ctx.enter_context(tc.tile_pool(name="py", bufs=3, space="PSUM"))
        prp = ctx.enter_context(tc.tile_pool(name="pr", bufs=1, space="PSUM"))
        pxp = ctx.enter_context(tc.tile_pool(name="px", bufs=4, space="PSUM"))

        ident = const.tile([128, 128], FP16)
        nc.sync.dma_start(ident[:], id_d.ap())
        mt = const.tile([128, NC128, NC128, 128], FP16)   # [p, jc, ic, q]
        wt = const.tile([128, NC128], FP32)
        ones = const.tile([128, 1], FP16)
        nc.vector.memset(ones[:], 1.0)

        # natural-layout batch tiles: [p, bn, i], row = bt*512 + bn*128 + p
        x_tiles = x_d.ap().rearrange("(t bn p) i -> t p bn i", p=128, bn=BT // 128)
        out_rows = out_d.ap().rearrange("(t b) one -> t one b", t=NBT)

        # HAM warmup: keep the PE busy through the initial DMA window so the
        # first real transposes/matmuls run closer to 2.4 GHz.
        wps = pyp.tile([128, 512], FP32, tag="py")
        for _ in range(30):
            nc.tensor.matmul(wps[:, :128], lhsT=ident[:], rhs=ident[:],
                             start=True, stop=True)

        x0_load = None
        for bt in range(NBT):
            # ---- x^T fp16 tiles via PE transposes; fp32->fp16 cast in-DMA
            # (reads 1MB, writes 0.5MB -- lowest SDMA traffic).  x0 gets
            # exclusive SDMA bandwidth; everything else dep-chains on it.
            xn16 = xn16p.tile([128, BT // 128, P], FP16)
            ld = nc.gpsimd.dma_start(xn16[:], x_tiles[bt])
            if bt == 0:
                m_src = m_d.ap().rearrange("jc p ic q -> p jc ic q")
                for dst, src in ((mt[:], m_src), (wt[:], w_d.ap())):
                    cld = nc.sync.dma_start(dst, src)
                    add_dep_helper(_raw_inst(cld), _raw_inst(ld),
                                   reason="give x0 exclusive bandwidth")
            else:
                # chain: each x tile load gets exclusive SDMA bandwidth and
                # completes before the tile that needs it next
                add_dep_helper(_raw_inst(ld), _raw_inst(x0_load),
                               reason="serialize x tile loads")
            x0_load = ld
            xt = xtp.tile([128, NC128, BT], FP16)
            for bn in range(BT // 128):
                # one single-bank PSUM tile per slab: no bank-overlap
                # serialization between transposes and the copy-out
                px = pxp.tile([128, NC128, 128], FP16)
                for ic in range(NC128):
                    nc.tensor.transpose(px[:, ic, :],
                                        xn16[:, bn, ts(ic, 128)], ident[:])
                nc.vector.tensor_copy(xt[:, :, ds(bn * 128, 128)], px[:])

            # ---- y'^T = M^T-chunks @ x^T ; z = (y'^T + w) * x^T ; reduce ----
            pr = prp.tile([1, BT], FP32)
            for jc in range(NC128):
                py = pyp.tile([128, BT], FP32)
                for ic in range(NC128):
                    nc.tensor.matmul(py[:], lhsT=mt[:, jc, ic, :],
                                     rhs=xt[:, ic, :],
                                     start=(ic == 0), stop=(ic == NC128 - 1))
                z = zp.tile([128, BT], FP16)
                nc.vector.scalar_tensor_tensor(
                    out=z[:], in0=py[:], scalar=wt[:, jc:jc + 1],
                    in1=xt[:, jc, :],
                    op0=mybir.AluOpType.add, op1=mybir.AluOpType.mult)
                nc.tensor.matmul(pr[:], lhsT=ones[:], rhs=z[:],
                                 start=(jc == 0), stop=(jc == NC128 - 1))

            orow = orp.tile([1, BT], FP32)
            nc.scalar.activation(orow[:], pr[:],
                                 mybir.ActivationFunctionType.Copy,
                                 bias=float(bias), scale=1.0)
            nc.sync.dma_start(out_rows[bt], orow[:])

    nc.compile()
    return nc


def kernel(x: np.ndarray, w: np.ndarray, v: np.ndarray, b: np.ndarray,
           f2f: np.ndarray) -> np.ndarray:
    global LAST_RESULT
    x = np.ascontiguousarray(np.asarray(x, dtype=np.float32))
    w = np.asarray(w, dtype=np.float32)
    v = np.asarray(v, dtype=np.float32)
    b = np.asarray(b, dtype=np.float32)
    f2f = np.asarray(f2f, dtype=np.int32)

    # ---- host: fold (v, f2f) into the interaction matrix M ----
    A = v[:, f2f, :]                                # [P, P, K]
    S = np.einsum('ijk,jik->ij', A, A)              # [P, P], symmetric
    M = 0.5 * (S - np.diag(np.diag(S)))             # strict-triu quadratic form

    # m_host[jc, p, ic, q] = M[ic*128 + p, jc*128 + q]
    m_host = np.ascontiguousarray(
        M.reshape(NC128, 128, NC128, 128).transpose(2, 1, 0, 3)
        .astype(np.float16))
    w_host = np.ascontiguousarray(
        w[:, 0].reshape(NC128, 128).T.astype(np.float32))  # [128, NC128]
    bias = float(b[0])

    nc = _build_nc(bias)

    ident_host = np.eye(128, dtype=np.float16)
    in_maps = []
    for c in range(N_CORES):
        in_maps.append({
            "x": np.ascontiguousarray(x[c * B_SH:(c + 1) * B_SH]),
            "m": m_host,
            "w": w_host,
            "ident": ident_host,
        })

    res = None
    last_exc = None
    for attempt in range(3):
        try:
            res = run_bass_kernel_spmd(nc, in_maps,
                                       core_ids=list(range(N_CORES)))
            break
        except Exception as exc:           # transient NRT/device hiccups
            last_exc = exc
            try:
                import jax
                jax.clear_caches()
                jax.extend.backend.clear_backends()
            except Exception:
                pass
            time.sleep(5.0)
    if res is None:
        raise last_exc
    LAST_RESULT = res

    out = np.concatenate([r["out"] for r in res.results], axis=0)
    return out.astype(np.float32)


if __name__ == "__main__":
    rng = np.random.default_rng(0)
    xs = rng.standard_normal((B, P), dtype=np.float32)
    ws = (rng.standard_normal((P, 1)) * 0.05).astype(np.float32)
    vs = (rng.standard_normal((P, F, K)) * 0.05).astype(np.float32)
    bs = rng.standard_normal((1,)).astype(np.float32)
    fs = rng.integers(0, F, size=(P,)).astype(np.int32)
    o = kernel(x=xs, w=ws, v=vs, b=bs, f2f=fs)
    print("out", o.shape, o.dtype, o[:4, 0])


# revision 58
# speedup vs baseline: 1.0728x; 1.0728x over previous
"""Trainium2 Bass kernel for an FFM (field-aware factorization machine) layer.

Reference computation (B=16384, P=512, F=16, K=8):
    A[i,j,:] = v[i, f2f[j], :]
    S[i,j]   = sum_k A[i,j,k] * A[j,i,k]          (symmetric)
    rp[b]    = sum_{i<j} x[b,i] * S[i,j] * x[b,j]
    out      = x @ w + rp[:,None] + b

Because S is symmetric, the strictly-upper-triangular quadratic form reduces to
    rp[b] = x[b] @ M @ x[b]^T,   M = 0.5 * (S - diag(S))
so with y' = x @ M + 1*w^T (a plain [512,512] matmul):
    out[b] = sum_j x[b,j] * (y'[b,j]) + bias

Host side folds (v, f2f, w) -> M (a tiny 512x512x8 einsum, ~0.4% of the FLOPs);
the device does the dominant work: the 16384x512x512 matmul, the elementwise
multiply and both reductions, data-parallel over batch across 8 NeuronCores.

Device kernel (per core, batch shard of 2048 rows), transposed orientation:
    x^T tiles produced on-chip: HBM fp32 load -> DVE cast fp16 -> XBAR DMA
    transpose.  y'^T[j,b] accumulated in PSUM from 4 K=128 fp16 matmuls with
    M-chunks stationary; a fused DVE scalar_tensor_tensor computes
    z = (y'^T + w) * x^T; a ones-vector matmul reduces z over partitions into
    rp^T; ACT adds the scalar bias; result DMAs out.
"""

import time
from contextlib import ExitStack

import numpy as np

import concourse.bass as bass
import concourse.mybir as mybir
import concourse.tile as tile
from concourse import bacc
from concourse.bass import ds, ts
from concourse.bass_utils import run_bass_kernel_spmd
from concourse.tile_rust import add_dep_helper


def _raw_inst(bass_inst):
    return getattr(bass_inst, "ins", bass_inst)

B, P, F, K = 16384, 512, 16, 8
N_CORES = 8
B_SH = B // N_CORES          # 2048 batch rows per core
BT = 512                     # batch tile (free dim of transposed tiles)
NBT = B_SH // BT             # 4 batch tiles per core
NC128 = P // 128             # 4 chunks of 128 along the feature dim

FP32 = mybir.dt.float32
FP16 = mybir.dt.float16

# test.py can read this after calling kernel() (exec_time_ns etc.)
LAST_RESULT = None


def _build_nc(bias: float) -> bass.Bass:
    nc = bacc.Bacc("TRN2", target_bir_lowering=False, debug=False,
                   num_devices=N_CORES)

    x_d = nc.dram_tensor("x", [B_SH, P], FP32, kind="ExternalInput")
    # m_d[jc, p, ic, q] = M[ic*128 + p, jc*128 + q]  (fp16, host-prepared)
    m_d = nc.dram_tensor("m", [NC128, 128, NC128, 128], FP16,
                         kind="ExternalInput")
    # w_d[p, c] = w[c*128 + p]
    w_d = nc.dram_tensor("w", [128, NC128], FP32, kind="ExternalInput")
    id_d = nc.dram_tensor("ident", [128, 128], FP16, kind="ExternalInput")
    out_d = nc.dram_tensor("out", [B_SH, 1], FP32, kind="ExternalOutput")

    with tile.TileContext(nc) as tc, ExitStack() as ctx:
        const = ctx.enter_context(tc.tile_pool(name="const", bufs=1))
        xn16p = ctx.enter_context(tc.tile_pool(name="xn16", bufs=3))
        xtp = ctx.enter_context(tc.tile_pool(name="xt", bufs=2))
        zp = ctx.enter_context(tc.tile_pool(name="z", bufs=3))
        orp = ctx.enter_context(tc.tile_pool(name="orow", bufs=2))
        pyp = ctx.enter_context(tc.tile_pool(name="py", bufs=2, space="PSUM"))
        prp = ctx.enter_context(tc.tile_pool(name="pr", bufs=1, space="PSUM"))
        pxp = ctx.enter_context(tc.tile_pool(name="px", bufs=4, space="PSUM"))

        ident = const.tile([128, 128], FP16)
        nc.sync.dma_start(ident[:], id_d.ap())
        mt = const.tile([128, NC128, NC128, 128], FP16)   # [p, jc, ic, q]
        wt = const.tile([128, NC128], FP32)
        ones = const.tile([128, 1], FP16)
        nc.vector.memset(ones[:], 1.0)

        # natural-layout batch tiles: [p, bn, i], row = bt*512 + bn*128 + p
        x_tiles = x_d.ap().rearrange("(t bn p) i -> t p bn i", p=128, bn=BT // 128)
        out_rows = out_d.ap().rearrange("(t b) one -> t one b", t=NBT)

        # HAM warmup: keep the PE busy through the initial DMA window so the
        # first real transposes/matmuls run closer to 2.4 GHz.
        wps = pyp.tile([128, 512], FP32, tag="py")
        for _ in range(30):
            nc.tensor.matmul(wps[:, :128], lhsT=ident[:], rhs=ident[:],
                             start=True, stop=True)

        x0_load = None
        for bt in range(NBT):
            # ---- x^T fp16 tiles via PE transposes; fp32->fp16 cast in-DMA
            # (reads 1MB, writes 0.5MB -- lowest SDMA traffic).  x0 gets
            # exclusive SDMA bandwidth; everything else dep-chains on it.
            xn16 = xn16p.tile([128, BT // 128, P], FP16)
            ld = nc.gpsimd.dma_start(xn16[:], x_tiles[bt])
            if bt == 0:
                m_src = m_d.ap().rearrange("jc p ic q -> p jc ic q")
                for dst, src in ((mt[:], m_src), (wt[:], w_d.ap())):
                    cld = nc.sync.dma_start(dst, src)
                    add_dep_helper(_raw_inst(cld), _raw_inst(ld),
                                   reason="give x0 exclusive bandwidth")
            else:
                # chain: each x tile load gets exclusive SDMA bandwidth and
                # completes before the tile that needs it next
                add_dep_helper(_raw_inst(ld), _raw_inst(x0_load),
                               reason="serialize x tile loads")
            x0_load = ld
            xt = xtp.tile([128, NC128, BT], FP16)
            for bn in range(BT // 128):
                # one single-bank PSUM tile per slab: no bank-overlap
                # serialization between transposes and the copy-out
                px = pxp.tile([128, NC128, 128], FP16)
                for ic in range(NC128):
                    nc.tensor.transpose(px[:, ic, :],
                                        xn16[:, bn, ts(ic, 128)], ident[:])
                nc.vector.tensor_copy(xt[:, :, ds(bn * 128, 128)], px[:])

            # ---- y'^T = M^T-chunks @ x^T ; z = (y'^T + w) * x^T ; reduce ----
            pr = prp.tile([1, BT], FP32)
            for jc in range(NC128):
                py = pyp.tile([128, BT], FP32)
                for ic in range(NC128):
                    nc.tensor.matmul(py[:], lhsT=mt[:, jc, ic, :],
                                     rhs=xt[:, ic, :],
                                     start=(ic == 0), stop=(ic == NC128 - 1))
                z = zp.tile([128, BT], FP16)
                nc.vector.scalar_tensor_tensor(
                    out=z[:], in0=py[:], scalar=wt[:, jc:jc + 1],
                    in1=xt[:, jc, :],
                    op0=mybir.AluOpType.add, op1=mybir.AluOpType.mult)
                nc.tensor.matmul(pr[:], lhsT=ones[:], rhs=z[:],
                                 start=(jc == 0), stop=(jc == NC128 - 1))

            orow = orp.tile([1, BT], FP32)
            nc.scalar.activation(orow[:], pr[:],
                                 mybir.ActivationFunctionType.Copy,
                                 bias=float(bias), scale=1.0)
            nc.sync.dma_start(out_rows[bt], orow[:])

    nc.compile()
    return nc


def kernel(x: np.ndarray, w: np.ndarray, v: np.ndarray, b: np.ndarray,
           f2f: np.ndarray) -> np.ndarray:
    global LAST_RESULT
    x = np.ascontiguousarray(np.asarray(x, dtype=np.float32))
    w = np.asarray(w, dtype=np.float32)
    v = np.asarray(v, dtype=np.float32)
    b = np.asarray(b, dtype=np.float32)
    f2f = np.asarray(f2f, dtype=np.int32)

    # ---- host: fold (v, f2f) into the interaction matrix M ----
    A = v[:, f2f, :]                                # [P, P, K]
    S = np.einsum('ijk,jik->ij', A, A)              # [P, P], symmetric
    M = 0.5 * (S - np.diag(np.diag(S)))             # strict-triu quadratic form

    # m_host[jc, p, ic, q] = M[ic*128 + p, jc*128 + q]
    m_host = np.ascontiguousarray(
        M.reshape(NC128, 128, NC128, 128).transpose(2, 1, 0, 3)
        .astype(np.float16))
    w_host = np.ascontiguousarray(
        w[:, 0].reshape(NC128, 128).T.astype(np.float32))  # [128, NC128]
    bias = float(b[0])

    nc = _build_nc(bias)

    ident_host = np.eye(128, dtype=np.float16)
    in_maps = []
    for c in range(N_CORES):
        in_maps.append({
            "x": np.ascontiguousarray(x[c * B_SH:(c + 1) * B_SH]),
            "m": m_host,
            "w": w_host,
            "ident": ident_host,
        })

    res = None
    last_exc = None
    for attempt in range(3):
        try:
            res = run_bass_kernel_spmd(nc, in_maps,
                                       core_ids=list(range(N_CORES)))
            break
        except Exception as exc:           # transient NRT/device hiccups
            last_exc = exc
            try:
                import jax
                jax.clear_caches()
                jax.extend.backend.clear_backends()
            except Exception:
                pass
            time.sleep(5.0)
    if res is None:
        raise last_exc
    LAST_RESULT = res

    out = np.concatenate([r["out"] for r in res.results], axis=0)
    return out.astype(np.float32)


if __name__ == "__main__":
    rng = np.random.default_rng(0)
    xs = rng.standard_normal((B, P), dtype=np.float32)
    ws = (rng.standard_normal((P, 1)) * 0.05).astype(np.float32)
    vs = (rng.standard_normal((P, F, K)) * 0.05).astype(np.float32)
    bs = rng.standard_normal((1,)).astype(np.float32)
    fs = rng.integers(0, F, size=(P,)).astype(np.int32)
    o = kernel(x=xs, w=ws, v=vs, b=bs, f2f=fs)
    print("out", o.shape, o.dtype, o[:4, 0])


# revision 59
# speedup vs baseline: 1.1014x; 1.0266x over previous
"""Trainium2 Bass kernel for an FFM (field-aware factorization machine) layer.

Reference computation (B=16384, P=512, F=16, K=8):
    A[i,j,:] = v[i, f2f[j], :]
    S[i,j]   = sum_k A[i,j,k] * A[j,i,k]          (symmetric)
    rp[b]    = sum_{i<j} x[b,i] * S[i,j] * x[b,j]
    out      = x @ w + rp[:,None] + b

Because S is symmetric, the strictly-upper-triangular quadratic form reduces to
    rp[b] = x[b] @ M @ x[b]^T,   M = 0.5 * (S - diag(S))
so with y' = x @ M + 1*w^T (a plain [512,512] matmul):
    out[b] = sum_j x[b,j] * (y'[b,j]) + bias

Host side folds (v, f2f, w) -> M (a tiny 512x512x8 einsum, ~0.4% of the FLOPs);
the device does the dominant work: the 16384x512x512 matmul, the elementwise
multiply and both reductions, data-parallel over batch across 8 NeuronCores.

Device kernel (per core, batch shard of 2048 rows), transposed orientation:
    x^T tiles produced on-chip: HBM fp32 load -> DVE cast fp16 -> XBAR DMA
    transpose.  y'^T[j,b] accumulated in PSUM from 4 K=128 fp16 matmuls with
    M-chunks stationary; a fused DVE scalar_tensor_tensor computes
    z = (y'^T + w) * x^T; a ones-vector matmul reduces z over partitions into
    rp^T; ACT adds the scalar bias; result DMAs out.
"""

import time
from contextlib import ExitStack

import numpy as np

import concourse.bass as bass
import concourse.mybir as mybir
import concourse.tile as tile
from concourse import bacc
from concourse.bass import ds, ts
from concourse.bass_utils import run_bass_kernel_spmd
from concourse.tile_rust import add_dep_helper


def _raw_inst(bass_inst):
    return getattr(bass_inst, "ins", bass_inst)

B, P, F, K = 16384, 512, 16, 8
N_CORES = 8
B_SH = B // N_CORES          # 2048 batch rows per core
BT = 512                     # batch tile (free dim of transposed tiles)
NBT = B_SH // BT             # 4 batch tiles per core
NC128 = P // 128             # 4 chunks of 128 along the feature dim

FP32 = mybir.dt.float32
FP16 = mybir.dt.float16

# test.py can read this after calling kernel() (exec_time_ns etc.)
LAST_RESULT = None


def _build_nc(bias: float) -> bass.Bass:
    nc = bacc.Bacc("TRN2", target_bir_lowering=False, debug=False,
                   num_devices=N_CORES)

    x_d = nc.dram_tensor("x", [B_SH, P], FP32, kind="ExternalInput")
    # m_d[jc, p, ic, q] = M[ic*128 + p, jc*128 + q]  (fp16, host-prepared)
    m_d = nc.dram_tensor("m", [NC128, 128, NC128, 128], FP16,
                         kind="ExternalInput")
    # w_d[p, c] = w[c*128 + p]
    w_d = nc.dram_tensor("w", [128, NC128], FP32, kind="ExternalInput")
    id_d = nc.dram_tensor("ident", [128, 128], FP16, kind="ExternalInput")
    out_d = nc.dram_tensor("out", [B_SH, 1], FP32, kind="ExternalOutput")

    with tile.TileContext(nc) as tc, ExitStack() as ctx:
        const = ctx.enter_context(tc.tile_pool(name="const", bufs=1))
        xn16p = ctx.enter_context(tc.tile_pool(name="xn16", bufs=3))
        xtp = ctx.enter_context(tc.tile_pool(name="xt", bufs=2))
        zp = ctx.enter_context(tc.tile_pool(name="z", bufs=3))
        orp = ctx.enter_context(tc.tile_pool(name="orow", bufs=2))
        pyp = ctx.enter_context(tc.tile_pool(name="py", bufs=2, space="PSUM"))
        prp = ctx.enter_context(tc.tile_pool(name="pr", bufs=1, space="PSUM"))
        pxp = ctx.enter_context(tc.tile_pool(name="px", bufs=4, space="PSUM"))

        ident = const.tile([128, 128], FP16)
        nc.sync.dma_start(ident[:], id_d.ap())
        mt = const.tile([128, NC128, NC128, 128], FP16)   # [p, jc, ic, q]
        wt = const.tile([128, NC128], FP32)
        ones = const.tile([128, 1], FP16)
        nc.vector.memset(ones[:], 1.0)

        # natural-layout batch tiles: [p, bn, i], row = bt*512 + bn*128 + p
        x_tiles = x_d.ap().rearrange("(t bn p) i -> t p bn i", p=128, bn=BT // 128)
        out_rows = out_d.ap().rearrange("(t b) one -> t one b", t=NBT)

        # HAM warmup: keep the PE busy through the initial DMA window so the
        # first real transposes/matmuls run closer to 2.4 GHz.
        wps = pyp.tile([128, 512], FP32, tag="py")
        for _ in range(30):
            nc.tensor.matmul(wps[:, :128], lhsT=ident[:], rhs=ident[:],
                             start=True, stop=True)

        x0_load = None
        for bt in range(NBT):
            # ---- x^T fp16 tiles via PE transposes; fp32->fp16 cast in-DMA
            # (reads 1MB, writes 0.5MB -- lowest SDMA traffic).  x0 gets
            # exclusive SDMA bandwidth; everything else dep-chains on it.
            xn16 = xn16p.tile([128, BT // 128, P], FP16)
            ld = nc.gpsimd.dma_start(xn16[:], x_tiles[bt])
            if bt == 0:
                m_src = m_d.ap().rearrange("jc p ic q -> p jc ic q")
                for dst, src in ((mt[:], m_src), (wt[:], w_d.ap())):
                    cld = nc.sync.dma_start(dst, src)
                    add_dep_helper(_raw_inst(cld), _raw_inst(ld),
                                   reason="give x0 exclusive bandwidth")
            else:
                add_dep_helper(_raw_inst(ld), _raw_inst(x0_load),
                               reason="give x0 exclusive bandwidth")
            if bt == 0:
                x0_load = ld
            xt = xtp.tile([128, NC128, BT], FP16)
            for bn in range(BT // 128):
                # one single-bank PSUM tile per slab: no bank-overlap
                # serialization between transposes and the copy-out
                px = pxp.tile([128, NC128, 128], FP16)
                for ic in range(NC128):
                    nc.tensor.transpose(px[:, ic, :],
                                        xn16[:, bn, ts(ic, 128)], ident[:])
                nc.vector.tensor_copy(xt[:, :, ds(bn * 128, 128)], px[:])

            # ---- y'^T = M^T-chunks @ x^T ; z = (y'^T + w) * x^T ; reduce ----
            pr = prp.tile([1, BT], FP32)
            for jc in range(NC128):
                py = pyp.tile([128, BT], FP32)
                for ic in range(NC128):
                    nc.tensor.matmul(py[:], lhsT=mt[:, jc, ic, :],
                                     rhs=xt[:, ic, :],
                                     start=(ic == 0), stop=(ic == NC128 - 1))
                z = zp.tile([128, BT], FP16)
                nc.vector.scalar_tensor_tensor(
                    out=z[:], in0=py[:], scalar=wt[:, jc:jc + 1],
                    in1=xt[:, jc, :],
                    op0=mybir.AluOpType.add, op1=mybir.AluOpType.mult)
                nc.tensor.matmul(pr[:], lhsT=ones[:], rhs=z[:],
                                 start=(jc == 0), stop=(jc == NC128 - 1))

            orow = orp.tile([1, BT], FP32)
            nc.scalar.activation(orow[:], pr[:],
                                 mybir.ActivationFunctionType.Copy,
                                 bias=float(bias), scale=1.0)
            nc.sync.dma_start(out_rows[bt], orow[:])

    nc.compile()
    return nc


def kernel(x: np.ndarray, w: np.ndarray, v: np.ndarray, b: np.ndarray,
           f2f: np.ndarray) -> np.ndarray:
    global LAST_RESULT
    x = np.ascontiguousarray(np.asarray(x, dtype=np.float32))
    w = np.asarray(w, dtype=np.float32)
    v = np.asarray(v, dtype=np.float32)
    b = np.asarray(b, dtype=np.float32)
    f2f = np.asarray(f2f, dtype=np.int32)

    # ---- host: fold (v, f2f) into the interaction matrix M ----
    A = v[:, f2f, :]                                # [P, P, K]
    S = np.einsum('ijk,jik->ij', A, A)              # [P, P], symmetric
    M = 0.5 * (S - np.diag(np.diag(S)))             # strict-triu quadratic form

    # m_host[jc, p, ic, q] = M[ic*128 + p, jc*128 + q]
    m_host = np.ascontiguousarray(
        M.reshape(NC128, 128, NC128, 128).transpose(2, 1, 0, 3)
        .astype(np.float16))
    w_host = np.ascontiguousarray(
        w[:, 0].reshape(NC128, 128).T.astype(np.float32))  # [128, NC128]
    bias = float(b[0])

    nc = _build_nc(bias)

    ident_host = np.eye(128, dtype=np.float16)
    in_maps = []
    for c in range(N_CORES):
        in_maps.append({
            "x": np.ascontiguousarray(x[c * B_SH:(c + 1) * B_SH]),
            "m": m_host,
            "w": w_host,
            "ident": ident_host,
        })

    res = None
    last_exc = None
    for attempt in range(3):
        try:
            res = run_bass_kernel_spmd(nc, in_maps,
                                       core_ids=list(range(N_CORES)))
            break
        except Exception as exc:           # transient NRT/device hiccups
            last_exc = exc
            try:
                import jax
                jax.clear_caches()
                jax.extend.backend.clear_backends()
            except Exception:
                pass
            time.sleep(5.0)
    if res is None:
        raise last_exc
    LAST_RESULT = res

    out = np.concatenate([r["out"] for r in res.results], axis=0)
    return out.astype(np.float32)


if __name__ == "__main__":
    rng = np.random.default_rng(0)
    xs = rng.standard_normal((B, P), dtype=np.float32)
    ws = (rng.standard_normal((P, 1)) * 0.05).astype(np.float32)
    vs = (rng.standard_normal((P, F, K)) * 0.05).astype(np.float32)
    bs = rng.standard_normal((1,)).astype(np.float32)
    fs = rng.integers(0, F, size=(P,)).astype(np.int32)
    o = kernel(x=xs, w=ws, v=vs, b=bs, f2f=fs)
    print("out", o.shape, o.dtype, o[:4, 0])


# revision 61
# speedup vs baseline: 1.1234x; 1.0199x over previous
"""Trainium2 Bass kernel for an FFM (field-aware factorization machine) layer.

Reference computation (B=16384, P=512, F=16, K=8):
    A[i,j,:] = v[i, f2f[j], :]
    S[i,j]   = sum_k A[i,j,k] * A[j,i,k]          (symmetric)
    rp[b]    = sum_{i<j} x[b,i] * S[i,j] * x[b,j]
    out      = x @ w + rp[:,None] + b

Because S is symmetric, the strictly-upper-triangular quadratic form reduces to
    rp[b] = x[b] @ M @ x[b]^T,   M = 0.5 * (S - diag(S))
so with y' = x @ M + 1*w^T (a plain [512,512] matmul):
    out[b] = sum_j x[b,j] * (y'[b,j]) + bias

Host side folds (v, f2f, w) -> M (a tiny 512x512x8 einsum, ~0.4% of the FLOPs);
the device does the dominant work: the 16384x512x512 matmul, the elementwise
multiply and both reductions, data-parallel over batch across 8 NeuronCores.

Device kernel (per core, batch shard of 2048 rows), transposed orientation:
    x^T tiles produced on-chip: HBM fp32 load -> DVE cast fp16 -> XBAR DMA
    transpose.  y'^T[j,b] accumulated in PSUM from 4 K=128 fp16 matmuls with
    M-chunks stationary; a fused DVE scalar_tensor_tensor computes
    z = (y'^T + w) * x^T; a ones-vector matmul reduces z over partitions into
    rp^T; ACT adds the scalar bias; result DMAs out.
"""

import time
from contextlib import ExitStack

import numpy as np

import concourse.bass as bass
import concourse.mybir as mybir
import concourse.tile as tile
from concourse import bacc
from concourse.bass import ds, ts
from concourse.bass_utils import run_bass_kernel_spmd
from concourse.tile_rust import add_dep_helper


def _raw_inst(bass_inst):
    return getattr(bass_inst, "ins", bass_inst)

B, P, F, K = 16384, 512, 16, 8
N_CORES = 8
B_SH = B // N_CORES          # 2048 batch rows per core
BT = 512                     # batch tile (free dim of transposed tiles)
NBT = B_SH // BT             # 4 batch tiles per core
NC128 = P // 128             # 4 chunks of 128 along the feature dim

FP32 = mybir.dt.float32
FP16 = mybir.dt.float16

# test.py can read this after calling kernel() (exec_time_ns etc.)
LAST_RESULT = None


def _build_nc(bias: float) -> bass.Bass:
    nc = bacc.Bacc("TRN2", target_bir_lowering=False, debug=False,
                   num_devices=N_CORES)

    x_d = nc.dram_tensor("x", [B_SH, P], FP32, kind="ExternalInput")
    # m_d[jc, p, ic, q] = M[ic*128 + p, jc*128 + q]  (fp16, host-prepared)
    m_d = nc.dram_tensor("m", [NC128, 128, NC128, 128], FP16,
                         kind="ExternalInput")
    # w_d[p, c] = w[c*128 + p]
    w_d = nc.dram_tensor("w", [128, NC128], FP32, kind="ExternalInput")
    id_d = nc.dram_tensor("ident", [128, 128], FP16, kind="ExternalInput")
    out_d = nc.dram_tensor("out", [B_SH, 1], FP32, kind="ExternalOutput")

    with tile.TileContext(nc) as tc, ExitStack() as ctx:
        const = ctx.enter_context(tc.tile_pool(name="const", bufs=1))
        xn16p = ctx.enter_context(tc.tile_pool(name="xn16", bufs=3))
        xtp = ctx.enter_context(tc.tile_pool(name="xt", bufs=2))
        zp = ctx.enter_context(tc.tile_pool(name="z", bufs=5))
        orp = ctx.enter_context(tc.tile_pool(name="orow", bufs=2))
        pyp = ctx.enter_context(tc.tile_pool(name="py", bufs=2, space="PSUM"))
        prp = ctx.enter_context(tc.tile_pool(name="pr", bufs=1, space="PSUM"))
        pxp = ctx.enter_context(tc.tile_pool(name="px", bufs=4, space="PSUM"))

        ident = const.tile([128, 128], FP16)
        nc.sync.dma_start(ident[:], id_d.ap())
        mt = const.tile([128, NC128, NC128, 128], FP16)   # [p, jc, ic, q]
        wt = const.tile([128, NC128], FP32)
        ones = const.tile([128, 1], FP16)
        nc.vector.memset(ones[:], 1.0)

        # natural-layout batch tiles: [p, bn, i], row = bt*512 + bn*128 + p
        x_tiles = x_d.ap().rearrange("(t bn p) i -> t p bn i", p=128, bn=BT // 128)
        out_rows = out_d.ap().rearrange("(t b) one -> t one b", t=NBT)

        # HAM warmup: keep the PE busy through the initial DMA window so the
        # first real transposes/matmuls run closer to 2.4 GHz.
        wps = pyp.tile([128, 512], FP32, tag="py")
        for _ in range(30):
            nc.tensor.matmul(wps[:, :128], lhsT=ident[:], rhs=ident[:],
                             start=True, stop=True)

        x0_load = None
        for bt in range(NBT):
            # ---- x^T fp16 tiles via PE transposes; fp32->fp16 cast in-DMA
            # (reads 1MB, writes 0.5MB -- lowest SDMA traffic).  x0 gets
            # exclusive SDMA bandwidth; everything else dep-chains on it.
            xn16 = xn16p.tile([128, BT // 128, P], FP16)
            ld = nc.gpsimd.dma_start(xn16[:], x_tiles[bt])
            if bt == 0:
                m_src = m_d.ap().rearrange("jc p ic q -> p jc ic q")
                for dst, src in ((mt[:], m_src), (wt[:], w_d.ap())):
                    cld = nc.sync.dma_start(dst, src)
                    add_dep_helper(_raw_inst(cld), _raw_inst(ld),
                                   reason="give x0 exclusive bandwidth")
            else:
                add_dep_helper(_raw_inst(ld), _raw_inst(x0_load),
                               reason="give x0 exclusive bandwidth")
            if bt == 0:
                x0_load = ld
            xt = xtp.tile([128, NC128, BT], FP16)
            for bn in range(BT // 128):
                # one single-bank PSUM tile per slab: no bank-overlap
                # serialization between transposes and the copy-out
                px = pxp.tile([128, NC128, 128], FP16)
                for ic in range(NC128):
                    nc.tensor.transpose(px[:, ic, :],
                                        xn16[:, bn, ts(ic, 128)], ident[:])
                nc.vector.tensor_copy(xt[:, :, ds(bn * 128, 128)], px[:])

            # ---- y'^T = M^T-chunks @ x^T ; z = (y'^T + w) * x^T ; reduce ----
            # All y-matmul groups are emitted before the four ones-reduce
            # matmuls so the PE never sits in FIFO order waiting on a z
            # that DVE has only just started.
            pr = prp.tile([1, BT], FP32)
            zs = []
            for jc in range(NC128):
                py = pyp.tile([128, BT], FP32)
                for ic in range(NC128):
                    nc.tensor.matmul(py[:], lhsT=mt[:, jc, ic, :],
                                     rhs=xt[:, ic, :],
                                     start=(ic == 0), stop=(ic == NC128 - 1))
                z = zp.tile([128, BT], FP16)
                nc.vector.scalar_tensor_tensor(
                    out=z[:], in0=py[:], scalar=wt[:, jc:jc + 1],
                    in1=xt[:, jc, :],
                    op0=mybir.AluOpType.add, op1=mybir.AluOpType.mult)
                zs.append(z)
            for jc, z in enumerate(zs):
                nc.tensor.matmul(pr[:], lhsT=ones[:], rhs=z[:],
                                 start=(jc == 0), stop=(jc == NC128 - 1))

            orow = orp.tile([1, BT], FP32)
            nc.scalar.activation(orow[:], pr[:],
                                 mybir.ActivationFunctionType.Copy,
                                 bias=float(bias), scale=1.0)
            nc.sync.dma_start(out_rows[bt], orow[:])

    nc.compile()
    return nc


def kernel(x: np.ndarray, w: np.ndarray, v: np.ndarray, b: np.ndarray,
           f2f: np.ndarray) -> np.ndarray:
    global LAST_RESULT
    x = np.ascontiguousarray(np.asarray(x, dtype=np.float32))
    w = np.asarray(w, dtype=np.float32)
    v = np.asarray(v, dtype=np.float32)
    b = np.asarray(b, dtype=np.float32)
    f2f = np.asarray(f2f, dtype=np.int32)

    # ---- host: fold (v, f2f) into the interaction matrix M ----
    A = v[:, f2f, :]                                # [P, P, K]
    S = np.einsum('ijk,jik->ij', A, A)              # [P, P], symmetric
    M = 0.5 * (S - np.diag(np.diag(S)))             # strict-triu quadratic form

    # m_host[jc, p, ic, q] = M[ic*128 + p, jc*128 + q]
    m_host = np.ascontiguousarray(
        M.reshape(NC128, 128, NC128, 128).transpose(2, 1, 0, 3)
        .astype(np.float16))
    w_host = np.ascontiguousarray(
        w[:, 0].reshape(NC128, 128).T.astype(np.float32))  # [128, NC128]
    bias = float(b[0])

    nc = _build_nc(bias)

    ident_host = np.eye(128, dtype=np.float16)
    in_maps = []
    for c in range(N_CORES):
        in_maps.append({
            "x": np.ascontiguousarray(x[c * B_SH:(c + 1) * B_SH]),
            "m": m_host,
            "w": w_host,
            "ident": ident_host,
        })

    res = None
    last_exc = None
    for attempt in range(3):
        try:
            res = run_bass_kernel_spmd(nc, in_maps,
                                       core_ids=list(range(N_CORES)))
            break
        except Exception as exc:           # transient NRT/device hiccups
            last_exc = exc
            try:
                import jax
                jax.clear_caches()
                jax.extend.backend.clear_backends()
            except Exception:
                pass
            time.sleep(5.0)
    if res is None:
        raise last_exc
    LAST_RESULT = res

    out = np.concatenate([r["out"] for r in res.results], axis=0)
    return out.astype(np.float32)


if __name__ == "__main__":
    rng = np.random.default_rng(0)
    xs = rng.standard_normal((B, P), dtype=np.float32)
    ws = (rng.standard_normal((P, 1)) * 0.05).astype(np.float32)
    vs = (rng.standard_normal((P, F, K)) * 0.05).astype(np.float32)
    bs = rng.standard_normal((1,)).astype(np.float32)
    fs = rng.integers(0, F, size=(P,)).astype(np.int32)
    o = kernel(x=xs, w=ws, v=vs, b=bs, f2f=fs)
    print("out", o.shape, o.dtype, o[:4, 0])
